# revision 1
# baseline (speedup 1.0000x reference)
"""Sparse (relu-cosine, causal+padding-masked) attention on 8 TRN2 NeuronCores.

Contract: kernel(**inputs) takes the full unsharded inputs and returns the
full [B, S, D] output. Internally:
  - host: compact each batch's tokens to the mask-valid ones (queries and
    keys share the same validity mask, so causal structure stays exactly
    lower-triangular in compacted space and all masking disappears),
    transpose X, slice per-head-pair weights, pad to tile multiples.
  - device (SPMD, 8 cores, 2 heads per core): QKV projections, cosine
    normalization folded into the relu scale (1/||k||) and a per-query
    broadcast tile (1/||q||), relu(QK^T) with triangular masks only on
    diagonal tiles, context accumulation (col-tiled pairs), and a partial
    output projection (transposed layout) through this core's 128 columns
    of Wo.
  - host: sum the 8 partial outputs, scatter rows back to the full
    [B, S, D] layout (masked query rows are exactly zero).

Matmul operands are bf16; every accumulation (PSUM) is fp32 and the
norm scales (1/||q||, 1/||k||) are computed from the fp32 sums.
1/||q|| is folded into the query vectors right after the projections
(scores are linear in q), and 1/||k|| into V, so the attention loop has
no normalization work left. Attention is software-pipelined per q-block:
scores of block i+1 are issued to the PE before the context matmuls of
block i, so the PE never waits on relu.

Perf notes (measured on HW): all large DMAs are single instructions
with >=1KB per-partition-contiguous descriptors (host pre-arranges the
DRAM layouts); every extra dma_start costs ~0.7us of serial DIRECT2D
issue on the sync sequencer, so only the 5 big loads go there and the
small consts use the scalar HWDGE queue. InstReciprocal costs ~2.6us
regardless of size - reciprocal_approx_fast is ~5x faster. fp8e4m3
(DoubleRow 2x matmul) was tried and reverted: each quantized stage
(att/vn/ctx/wot) alone adds ~2.3-3.3%% fro error vs the 2%% budget.
"""

import numpy as np

B, S, D, H = 2, 2048, 1024, 16
DH = D // H
NCORES = 8
HEADS_PER_CORE = H // NCORES  # 2
NH = HEADS_PER_CORE
JW = HEADS_PER_CORE * DH  # 128, per-core head-dim slice width
QB = 512  # query block width (one fp32 PSUM bank)
KT = 128  # key tile (partition dim)


def _build_program(LQs, n_dblk=D // 128):
    import concourse.bass as bass
    import concourse.mybir as mybir
    import concourse.tile as tile
    from concourse import bacc
    from concourse.bass import ts
    from concourse.masks import make_identity

    F32 = mybir.dt.float32
    F32R = mybir.dt.float32r
    F16 = mybir.dt.float16
    BF16 = mybir.dt.bfloat16
    F8E4 = mybir.dt.float8e4
    DROW = mybir.MatmulPerfMode.DoubleRow
    AF = mybir.ActivationFunctionType
    MULT = mybir.AluOpType.mult
    MAX = mybir.AluOpType.max

    LT = sum(LQs)
    offs = [0, LQs[0]]
    n_ttiles = LT // 128
    NBLK = -(-LT // QB)
    LT_pad = NBLK * QB

    nc = bacc.Bacc("TRN2", target_bir_lowering=False, debug=False,
                   num_devices=NCORES)

    # all big DRAM tensors are host-prearranged so every DMA is one
    # instruction with large per-partition-contiguous descriptors (one
    # dma_start already spreads over all 16 SDMA engines; many small
    # dma_starts just pay ~0.7us DIRECT2D issue cost each on the sync
    # sequencer).
    XT = nc.dram_tensor("XT", [NBLK, 128, n_dblk * QB], BF16,
                        kind="ExternalInput").ap()
    # q/k/v weights batched into one DMA: [128, 3, n_dblk*JW]
    WALL = nc.dram_tensor("WALL", [128, 3 * n_dblk * JW], BF16,
                          kind="ExternalInput").ap()
    WOT = nc.dram_tensor("WOT", [JW, D], BF16, kind="ExternalInput").ap()
    # causal diag-tile masks: CAUS[:, 384-off : 384-off+qbw], off = kt0-q0
    CAUS = nc.dram_tensor("CAUS", [128, 896], BF16, kind="ExternalInput").ap()
    # IND[j, h] = 1 if j // DH == h ; INDT is its transpose
    IND = nc.dram_tensor("IND", [JW, NH], BF16, kind="ExternalInput").ap()
    INDT = nc.dram_tensor("INDT", [NH, JW], F32R, kind="ExternalInput").ap()
    # transposed output, per-q-block contiguous; host reassembles
    n_qblocks = sum(-(-lq // QB) for lq in LQs)
    OUTT = nc.dram_tensor("OUTT", [n_qblocks, 128, n_dblk * QB], F16,
                          kind="ExternalOutput").ap()

    EPS = 1e-12

    def col_blocks(width, bw=QB):
        blocks = []
        c = 0
        while c < width:
            w = min(bw, width - c)
            blocks.append((c, w))
            c += w
        return blocks

    with tile.TileContext(nc) as tc:
        with (
            tc.tile_pool(name="consts", bufs=1) as consts,
            tc.tile_pool(name="proj", bufs=1) as projp,
            tc.tile_pool(name="work", bufs=3) as work,
            tc.tile_pool(name="outp", bufs=2) as outp,
            tc.tile_pool(name="ps_mm", bufs=1, space="PSUM") as ps_mm,
            tc.tile_pool(name="ps_ctxp", bufs=1, space="PSUM") as ps_ctxp,
            tc.tile_pool(name="ps_scp", bufs=6, space="PSUM") as ps_scp,
        ):
            # ---- weights first (first projection matmul needs them); the
            # sync sequencer issues [wq, xt0, wkv, xt1..] only, in that
            # order: the first Q-projection matmul needs just Wq + xt block
            # 0, so those two transfers go ahead of everything else and the
            # small consts use the scalar HWDGE queue.
            wall = consts.tile([128, 3, n_dblk, JW], BF16)
            xtp = tc.alloc_tile_pool(name="xt", bufs=1)
            xt = xtp.tile([128, NBLK, n_dblk, QB], BF16)

            # X^T block-major: each DMA is 1MB of 8KB-contiguous runs. The
            # first Q-projection matmul needs only Wq d-chunks 0-3 and xt0
            # d-chunks 0-1, so those ship first in small transfers; the PE
            # start is data-gated, and this trims the gate to ~380KB.
            wq_half = n_dblk // 2 * JW
            wqv = WALL[:, :n_dblk * JW].rearrange("p (k j) -> p k j",
                                                  k=n_dblk)
            src0 = XT[0].rearrange("p (k w) -> p k w", k=n_dblk)
            half = n_dblk // 2
            nc.sync.dma_start(out=wall[:, 0, :half], in_=wqv[:, :half])
            nc.sync.dma_start(out=xt[:, 0, :2], in_=src0[:, :2])
            nc.sync.dma_start(out=wall[:, 0, half:], in_=wqv[:, half:])
            nc.sync.dma_start(out=xt[:, 0, 2:], in_=src0[:, 2:])
            nc.sync.dma_start(
                out=wall[:, 1:],
                in_=WALL[:, n_dblk * JW:].rearrange("p (s k j) -> p s k j",
                                                    s=2, k=n_dblk))
            for b in range(1, NBLK):
                nc.sync.dma_start(
                    out=xt[:, b],
                    in_=XT[b].rearrange("p (k w) -> p k w", k=n_dblk))
            wqt, wkt, wvt = wall[:, 0], wall[:, 1], wall[:, 2]

            # ---- remaining constants (scalar HWDGE queue) ------------------
            # caus/wot allocs here, but their DMAs are deferred to after
            # projection block 0: their 0.5MB would otherwise compete with
            # the startup-critical Wq/xt0 delivery, and their consumers
            # (relu masks, out-projection) run tens of us later.
            caus = consts.tile([128, 896], BF16)
            wot = consts.tile([JW, D], BF16)
            ind = consts.tile([JW, NH], BF16)
            nc.scalar.dma_start(out=ind, in_=IND[:, :])
            indt = consts.tile([NH, JW], F32R)
            nc.scalar.dma_start(out=indt, in_=INDT[:, :])
            eps128 = consts.tile([128, 1], F32)
            nc.vector.memset(eps128, EPS)
            ident = consts.tile([128, 128], BF16)
            make_identity(nc, ident)

            # ---- projections ------------------------------------------------
            # qt/kt/vt in bf16; squares for the norms are taken from the f32
            # PSUM so the scales stay exact for the rounded Q/K.
            qt = projp.tile([JW, LT], BF16)
            kt_ = projp.tile([JW, LT], BF16)
            vt = projp.tile([JW, LT], BF16)
            qsq = projp.tile([JW, LT], BF16)
            ksq = projp.tile([JW, LT], BF16)
            ksc = projp.tile([128, n_ttiles, NH], F32)
            vn = projp.tile([128, n_ttiles, JW], BF16)

            # qt *= 1/|q| (scores are linear in q; qsb partition p carries
            # head(p)'s scale, matching qt's layout). The chain for block g
            # (sqrt on scalar, reciprocal+multiply on vector) overlaps the
            # projection matmuls of block g+1, so the PE never waits on it:
            # the qsum matmul is emitted with proj(g), the rest one block
            # later via flush_norm.
            norm_pend = []

            def flush_norm():
                c0, w, ssq = norm_pend.pop(0)
                ps_qsb = ps_ctxp.tile([128, QB], F32, tag="ctx_ps",
                                      name="ps_qsb")
                nc.tensor.matmul(ps_qsb[:, :w], indt[:, :], ssq[:, :w],
                                 start=True, stop=True)
                qsb = work.tile([128, QB], F32, tag="qsb")
                nc.vector.reciprocal_approx_fast(out=qsb[:, :w],
                                                 in_=ps_qsb[:, :w])
                # SBUF-only multiply on gpsimd: keeps vector's queue clear
                # for the first relus (the consumer, scores(g), is far away)
                nc.gpsimd.tensor_mul(qt[:, c0:c0 + w], qt[:, c0:c0 + w],
                                     qsb[:, :w])

            cp_i = 0
            for bi, (c0, w) in enumerate(col_blocks(LT)):
                for dst, wmat, sq in ((qt, wqt, qsq), (kt_, wkt, ksq),
                                      (vt, wvt, None)):
                    ps = ps_scp.tile([JW, QB], F32, tag="sc", name="ps_proj")
                    for k in range(n_dblk):
                        nc.tensor.matmul(
                            ps[:, :w], wmat[:, k, :], xt[:, bi, k, :w],
                            start=(k == 0), stop=(k == n_dblk - 1),
                        )
                    if cp_i % 2 == 0:
                        nc.vector.tensor_copy(dst[:, c0:c0 + w], ps[:, :w])
                    else:
                        nc.scalar.activation(out=dst[:, c0:c0 + w],
                                             in_=ps[:, :w], func=AF.Copy)
                    cp_i += 1
                    if sq is not None:
                        # gpsimd is otherwise idle and these are SBUF->SBUF
                        nc.gpsimd.tensor_mul(sq[:, c0:c0 + w],
                                             dst[:, c0:c0 + w],
                                             dst[:, c0:c0 + w])
                ps_ss = ps_mm.tile([NH, QB], F32, tag="mm", name="ps_qsum")
                nc.tensor.matmul(ps_ss[:, :w], ind[:, :], qsq[:, c0:c0 + w],
                                 start=True, stop=True)
                ssq = work.tile([NH, QB], F32R, tag="ssq")
                nc.scalar.activation(out=ssq[:, :w], in_=ps_ss[:, :w],
                                     func=AF.Sqrt, bias=eps128[:NH, :],
                                     scale=1.0)
                norm_pend.append((c0, w, ssq))
                if len(norm_pend) > 1:
                    flush_norm()
                if bi == 0:
                    nc.scalar.dma_start(out=caus, in_=CAUS[:, :])
                    nc.scalar.dma_start(out=wot, in_=WOT[:, :])

            # ---- kscale[t, h] = rsqrt(sum_j ksq[j, t] over head h) ----------
            # (batched after the projections: emitting these per proj block
            # was tried and regressed — the chains serialize on the PE's
            # path through the single-buffer PSUM zone recycling)
            ksum_ps = ps_ctxp.tile([128, n_ttiles * NH], F32, tag="ctx_ps",
                                   name="ksum_ps")
            for tt in range(n_ttiles):
                nc.tensor.matmul(ksum_ps[:, tt * NH:(tt + 1) * NH],
                                 ksq[:, ts(tt, 128)], ind[:, :],
                                 start=True, stop=True, skip_group_check=True)
            nc.scalar.activation(out=ksc[:, :, :].rearrange("p a b -> p (a b)"),
                                 in_=ksum_ps[:, :], func=AF.Sqrt,
                                 bias=eps128[:, :], scale=1.0)
            nc.vector.reciprocal_approx_fast(
                out=ksc[:, :, :].rearrange("p a b -> p (a b)"),
                in_=ksc[:, :, :].rearrange("p a b -> p (a b)"))
            # (the last block's 1/|q| flush happens in the driver, after the
            # first block's scores: ps_qsb may only recycle the ksum bank
            # once its reader, the ksc sqrt above, has been emitted)

            def emit_vn(t_lo, t_hi):
                for tt in range(t_lo, t_hi):
                    ps = ps_scp.tile([128, 128], BF16, tag="sc",
                                     name="ps_vtr")
                    nc.tensor.transpose(ps[:, :], vt[:, ts(tt, 128)], ident)
                    if tt % 2 == 0:
                        # both heads in one op: broadcast each head's scale
                        # over its 64 columns via a 0-stride dim
                        kv = ksc[:, tt, :]
                        kv2 = bass.AP(tensor=kv.tensor, offset=kv.offset,
                                      ap=[kv.ap[0], kv.ap[1], [0, DH]])
                        nc.vector.tensor_mul(
                            vn[:, tt, :].rearrange("p (h j) -> p h j", h=NH),
                            ps[:, :].rearrange("p (h j) -> p h j", h=NH),
                            kv2)
                    else:
                        for h in range(NH):
                            nc.scalar.activation(
                                out=vn[:, tt, ts(h, DH)],
                                in_=ps[:, ts(h, DH)],
                                func=AF.Copy, scale=ksc[:, tt, h:h + 1])

            # keep xt resident: releasing it here would make the att pool
            # reuse its SBUF zone, serializing attention start behind the
            # last projection matmul. Both fit in SBUF at bf16 sizes.
            max_nkt = max(LQs) // KT
            att_bufs = 3 if max_nkt <= 10 else (2 if max_nkt <= 14 else 1)
            attp = tc.alloc_tile_pool(name="att", bufs=att_bufs)

            # ---- attention blocks, software-pipelined over q-blocks --------
            blocks = []
            for b in range(B):
                for q0, qw in col_blocks(LQs[b]):
                    blocks.append((b, q0, qw))
            ctx_sbs = {
                b: attp.tile([JW, LQs[b]], BF16, tag=f"ctx_{b}", bufs=1,
                             name=f"ctx_sb{b}")
                for b in range(B)
            }

            state = {}

            def emit_scores(blk):
                b, q0, qw = blk
                ob = offs[b]
                lq = LQs[b]
                n_kt = min((q0 + qw + KT - 1) // KT, lq // KT)
                att_sb = attp.tile([128, max_nkt * NH, QB], BF16,
                                   tag="att_sb", name="att_sb")
                offs_ki = []
                diag_i = 0
                relu_i = 0
                for ki in range(n_kt):
                    k0 = ki * KT
                    # columns < off are fully masked by causality; skip them
                    off = max(0, k0 - q0)
                    offs_ki.append(off)
                    w = qw - off
                    diag = k0 > q0 - KT
                    # per-head 1-bank score tiles: with every ps_scp tile
                    # at most one bank, the pool holds 6 rotation slots in
                    # the same 6 banks, doubling the pipeline depth that
                    # scores/outproj allocations wait on
                    for h in range(NH):
                        sc_h = ps_scp.tile([128, QB], F32, tag="sc",
                                           name="sc_ps")
                        nc.tensor.matmul(
                            sc_h[:, off:qw],
                            kt_[ts(h, DH), ob + k0:ob + k0 + KT],
                            qt[ts(h, DH), ob + q0 + off:ob + q0 + qw],
                            start=True, stop=True,
                        )
                        slot = att_sb[:, ki * NH + h, off:qw]
                        if diag:
                            cs = caus[:, 384:384 + w]
                            if (diag_i + h) % 2 == 0:
                                nc.vector.scalar_tensor_tensor(
                                    out=slot, in0=sc_h[:, off:qw],
                                    scalar=0.0, in1=cs, op0=MAX, op1=MULT)
                            else:
                                nc.scalar.activation(
                                    out=slot, in_=sc_h[:, off:qw],
                                    func=AF.Relu)
                                nc.vector.tensor_mul(slot, slot, cs)
                        else:
                            if (relu_i + h) % 2 == 1:
                                nc.scalar.activation(
                                    out=slot, in_=sc_h[:, off:qw],
                                    func=AF.Relu)
                            else:
                                nc.vector.tensor_scalar_max(
                                    out=slot, in0=sc_h[:, off:qw],
                                    scalar1=0.0)
                    if diag:
                        diag_i += 1
                    else:
                        relu_i += 1
                state[blk] = (att_sb, n_kt, offs_ki)

            def emit_ctx(blk_i, blk):
                b, q0, qw = blk
                ob = offs[b]
                ctx_sb = ctx_sbs[b]
                att_sb, n_kt, offs_ki = state.pop(blk)
                # col-tiled pair: both heads accumulate in one PSUM bank.
                # the final block borrows the (idle by then) ps_mm bank so
                # its matmuls don't wait on the previous block's ctx copy.
                if blk_i == n_qblocks - 1:
                    ctx_ps = ps_mm.tile([128, QB], F32, tag="mm",
                                        name="ctx_ps_last")
                else:
                    ctx_ps = ps_ctxp.tile([128, QB], F32, tag="ctx_ps",
                                          name="ctx_ps")
                assert offs_ki[0] == 0  # first tile always starts the bank
                for ki in range(n_kt):
                    gtt = (ob + ki * KT) // KT
                    off = offs_ki[ki]
                    for h in range(NH):
                        nc.tensor.matmul(
                            ctx_ps[ts(h, DH), off:qw],
                            vn[:, gtt, ts(h, DH)],
                            att_sb[:, ki * NH + h, off:qw],
                            start=(ki == 0), stop=(ki == n_kt - 1),
                            tile_position=(0, h * DH),
                            skip_group_check=True,
                        )
                # 1/|q| already folded into the queries; plain copy. Scalar
                # mid-stream (vector carries the relus there); vector for the
                # last two blocks, where it is idle and scalar still has the
                # previous block's out-copies queued.
                if blk_i >= n_qblocks - 2:
                    nc.vector.tensor_copy(ctx_sb[:, q0:q0 + qw],
                                          ctx_ps[:, :qw])
                else:
                    nc.scalar.activation(out=ctx_sb[:, q0:q0 + qw],
                                         in_=ctx_ps[:, :qw], func=AF.Copy)

            def emit_outproj(blk_i, blk):
                b, q0, qw = blk
                ob = offs[b]
                ctx_sb = ctx_sbs[b]
                # output projection (transposed layout), this q-block only;
                # per-chunk 1-bank PSUM tiles (see scores); all 8 dout
                # chunks gather into one SBUF tile and ship in a single DMA.
                o_all = outp.tile([128, n_dblk, QB], F16, tag="o_sb")
                dst = OUTT[blk_i].rearrange("p (g w) -> p g w", g=n_dblk)
                for g in range(n_dblk):
                    ps = ps_scp.tile([128, QB], F32, tag="sc", name="ps_out")
                    nc.tensor.matmul(ps[:, :qw], wot[:, ts(g, 128)],
                                     ctx_sb[:, q0:q0 + qw],
                                     start=True, stop=True)
                    if g % 2 == 0:
                        nc.vector.tensor_copy(o_all[:, g, :qw], ps[:, :qw])
                    else:
                        nc.scalar.activation(out=o_all[:, g, :qw],
                                             in_=ps[:, :qw], func=AF.Copy)
                nc.sync.dma_start(out=dst[:, :, :qw], in_=o_all[:, :, :qw])

            def emit_outproj_pair(i1, blk1, i2, blk2):
                # drain: interleave the two blocks' dout chunks so each
                # block's PSUM->SBUF copies retire under the other block's
                # matmuls instead of stalling the PE via pool rotation
                parts = []
                for blk_i, (b, q0, qw) in ((i1, blk1), (i2, blk2)):
                    o_all = outp.tile([128, n_dblk, QB], F16, tag="o_sb")
                    dst = OUTT[blk_i].rearrange("p (g w) -> p g w", g=n_dblk)
                    parts.append((ctx_sbs[b], q0, qw, o_all, dst))
                for g in range(n_dblk):
                    for pi, (ctx_sb, q0, qw, o_all, dst) in enumerate(parts):
                        ps = ps_scp.tile([128, QB], F32, tag="sc",
                                         name="ps_out")
                        nc.tensor.matmul(ps[:, :qw], wot[:, ts(g, 128)],
                                         ctx_sb[:, q0:q0 + qw],
                                         start=True, stop=True)
                        if (g + pi) % 2 == 0:
                            nc.vector.tensor_copy(o_all[:, g, :qw],
                                                  ps[:, :qw])
                        else:
                            nc.scalar.activation(out=o_all[:, g, :qw],
                                                 in_=ps[:, :qw],
                                                 func=AF.Copy)
                        if g == n_dblk - 3:
                            # overlap most of the final stores with the
                            # last chunks' compute
                            nc.sync.dma_start(out=dst[:, :g + 1, :qw],
                                              in_=o_all[:, :g + 1, :qw])
                g0 = n_dblk - 2
                for ctx_sb, q0, qw, o_all, dst in parts:
                    nc.sync.dma_start(out=dst[:, g0:, :qw],
                                      in_=o_all[:, g0:, :qw])

            def emit_ctx_out(blk_i, blk):
                emit_ctx(blk_i, blk)
                emit_outproj(blk_i, blk)

            # depth-2 software pipeline: scores of blocks i and i+1 are both
            # in flight before ctx of block i-1, giving the relu/copy engines
            # a full extra block of slack before the PE consumes their output.
            # scores(0) goes FIRST: it needs no vn, so its matmuls keep the
            # PE fed while the ksc/vn scale chains drain on vector/scalar.
            emit_scores(blocks[0])
            flush_norm()  # last block's qt scale, behind the first relus
            # vn in two halves around scores(1): spreads the 24 small scale
            # ops over two windows so relu(0)/relu(1) aren't queued behind
            # them all at once
            emit_vn(0, n_ttiles // 2)
            if len(blocks) > 1:
                emit_scores(blocks[1])
            emit_vn(n_ttiles // 2, n_ttiles)
            for i, blk in enumerate(blocks):
                if i <= 1:
                    continue
                emit_scores(blk)
                if i > 1:
                    emit_ctx_out(i - 2, blocks[i - 2])
            # drain: interleave the last two blocks' ctx and out-projection
            # so each out-projection's ctx copy completes under the other
            # block's matmuls instead of stalling the PE
            n = len(blocks)
            if n > 1:
                emit_ctx(n - 2, blocks[-2])
                emit_ctx(n - 1, blocks[-1])
                emit_outproj_pair(n - 2, blocks[-2], n - 1, blocks[-1])
            else:
                emit_ctx_out(0, blocks[0])
            attp.release()
            xtp.release()

    nc.compile()
    return nc


def _prepare(X, masks, Wq, Wk, Wv, Wo):
    import ml_dtypes
    BF = ml_dtypes.bfloat16
    F8 = ml_dtypes.float8_e4m3

    X = np.asarray(X, dtype=np.float32)
    masks = np.asarray(masks)
    Wq = np.asarray(Wq, dtype=np.float32)
    Wk = np.asarray(Wk, dtype=np.float32)
    Wv = np.asarray(Wv, dtype=np.float32)
    Wo = np.asarray(Wo, dtype=np.float32)

    idxs = [np.where(masks[b] != 0)[0] for b in range(B)]
    # 256-multiples so fp8 DoubleRow k-tile pairs align for both batches
    LQs = [max(256, int(-(-len(ix) // 256) * 256)) for ix in idxs]
    LT = sum(LQs)
    offs = [0, LQs[0]]
    QBK = 512
    NBLK = -(-LT // QBK)
    LT_pad = NBLK * QBK
    n_dblk = D // 128

    # compacted, transposed X: columns = valid tokens (zero-padded)
    XTc = np.zeros((D, LT_pad), dtype=np.float32)
    for b in range(B):
        XTc[:, offs[b]:offs[b] + len(idxs[b])] = X[b].T[:, idxs[b]]
    # DMA-friendly: [NBLK, 128, n_dblk*QBK], per-partition contiguous
    XTa = np.ascontiguousarray(
        XTc.reshape(n_dblk, 128, NBLK, QBK).transpose(2, 1, 0, 3)
        .reshape(NBLK, 128, n_dblk * QBK)).astype(BF)

    caus = (np.arange(896)[None, :] - 384 >= np.arange(128)[:, None])

    nc = _build_program(LQs)

    def warr(wT):  # [D, JW] -> [128, n_dblk*JW] per-partition contiguous
        return np.ascontiguousarray(
            wT.reshape(n_dblk, 128, JW).transpose(1, 0, 2)
            .reshape(128, n_dblk * JW)).astype(BF)

    in_maps = []
    for c in range(NCORES):
        jsl = slice(c * JW, (c + 1) * JW)
        ind = np.zeros((JW, NH), dtype=np.float32)
        for h in range(NH):
            ind[h * DH:(h + 1) * DH, h] = 1.0
        in_maps.append({
            "XT": XTa,
            "WALL": np.ascontiguousarray(np.concatenate(
                [warr(Wq[jsl, :].T), warr(Wk[jsl, :].T),
                 warr(Wv[jsl, :].T)], axis=1)),
            "WOT": np.ascontiguousarray(Wo[:, jsl].T).astype(BF),
            "CAUS": caus.astype(BF),
            "IND": ind.astype(BF),
            "INDT": np.ascontiguousarray(ind.T),
        })

    return nc, in_maps, (idxs, LQs, LT, offs)


def _unshard(results, meta):
    idxs, LQs, LT, offs = meta
    n_dblk = D // 128
    blocks = []
    for b in range(B):
        q0 = 0
        while q0 < LQs[b]:
            qw = min(QB, LQs[b] - q0)
            blocks.append((b, q0, qw))
            q0 += qw

    partial = np.zeros((D, LT), dtype=np.float64)
    for c in range(NCORES):
        # OUTT[i, p, g*QB + w] = out[g*128 + p, ob + q0 + w] for block i
        ot = results[c]["OUTT"].astype(np.float64).reshape(
            len(blocks), 128, n_dblk, QB)
        for i, (b, q0, qw) in enumerate(blocks):
            cols = slice(offs[b] + q0, offs[b] + q0 + qw)
            partial[:, cols] += ot[i, :, :, :qw].transpose(1, 0, 2).reshape(
                D, qw)
    partial = partial.T  # [LT, D]

    out = np.zeros((B, S, D), dtype=np.float32)
    for b in range(B):
        out[b, idxs[b], :] = partial[offs[b]:offs[b] + len(idxs[b]), :].astype(
            np.float32)
    return out


def kernel(X, masks, Wq, Wk, Wv, Wo):
    from concourse.bass_utils import run_bass_kernel_spmd

    nc, in_maps, meta = _prepare(X, masks, Wq, Wk, Wv, Wo)
    res = run_bass_kernel_spmd(nc, in_maps, list(range(NCORES)))
    return _unshard(res.results, meta)


def profile_run(inputs, tmpdir=None):
    """Used by test.py: same program, run with NTFF tracing enabled."""
    from concourse.bass_utils import run_bass_kernel_spmd

    nc, in_maps, meta = _prepare(**inputs)
    res = run_bass_kernel_spmd(nc, in_maps, list(range(NCORES)), trace=True,
                               tmpdir=tmpdir)
    res.output = _unshard(res.results, meta)
    return res



# revision 13
# speedup vs baseline: 1.2096x; 1.2096x over previous
"""Sparse (relu-cosine, causal+padding-masked) attention on 8 TRN2 NeuronCores.

Contract: kernel(**inputs) takes the full unsharded inputs and returns the
full [B, S, D] output. Internally:
  - host: compact each batch's tokens to the mask-valid ones (queries and
    keys share the same validity mask, so causal structure stays exactly
    lower-triangular in compacted space and all masking disappears),
    transpose X, slice per-head-pair weights, pad to tile multiples.
  - device (SPMD, 8 cores, 2 heads per core): QKV projections, cosine
    normalization folded into the relu scale (1/||k||) and a per-query
    broadcast tile (1/||q||), relu(QK^T) with triangular masks only on
    diagonal tiles, context accumulation (col-tiled pairs), and a partial
    output projection (transposed layout) through this core's 128 columns
    of Wo.
  - host: sum the 8 partial outputs, scatter rows back to the full
    [B, S, D] layout (masked query rows are exactly zero).

Matmul operands are bf16; every accumulation (PSUM) is fp32 and the
norm scales (1/||q||, 1/||k||) are computed from the fp32 sums.
1/||q|| is folded into the query vectors right after the projections
(scores are linear in q), and 1/||k|| into V, so the attention loop has
no normalization work left. Attention is software-pipelined per q-block:
scores of block i+1 are issued to the PE before the context matmuls of
block i, so the PE never waits on relu.

Perf notes (measured on HW): all large DMAs are single instructions
with >=1KB per-partition-contiguous descriptors (host pre-arranges the
DRAM layouts); every extra dma_start costs ~0.7us of serial DIRECT2D
issue on the sync sequencer, so only the 5 big loads go there and the
small consts use the scalar HWDGE queue. InstReciprocal costs ~2.6us
regardless of size - reciprocal_approx_fast is ~5x faster. fp8e4m3
(DoubleRow 2x matmul) was tried and reverted: each quantized stage
(att/vn/ctx/wot) alone adds ~2.3-3.3%% fro error vs the 2%% budget.
"""

import numpy as np

B, S, D, H = 2, 2048, 1024, 16
DH = D // H
NCORES = 8
HEADS_PER_CORE = H // NCORES  # 2
NH = HEADS_PER_CORE
JW = HEADS_PER_CORE * DH  # 128, per-core head-dim slice width
QB = 512  # query block width (one fp32 PSUM bank)
KT = 128  # key tile (partition dim)


def _col_blocks(width, bw=QB):
    blocks = []
    c = 0
    while c < width:
        w = min(bw, width - c)
        blocks.append((c, w))
        c += w
    return blocks


def _block_order(LQs):
    """Batch-interleaved q-block order (b0B0, b1B0, b0B1, ...)."""
    per = [_col_blocks(lq) for lq in LQs]
    out = []
    for j in range(max(len(p) for p in per)):
        for b in range(len(LQs)):
            if j < len(per[b]):
                out.append((b, per[b][j][0], per[b][j][1]))
    return out


def _build_program(LQs, n_dblk=D // 128):
    import concourse.bass as bass
    import concourse.mybir as mybir
    import concourse.tile as tile
    from concourse import bacc
    from concourse.bass import ts

    F32 = mybir.dt.float32
    F32R = mybir.dt.float32r
    F16 = mybir.dt.float16
    BF16 = mybir.dt.bfloat16
    F8E4 = mybir.dt.float8e4
    DROW = mybir.MatmulPerfMode.DoubleRow
    AF = mybir.ActivationFunctionType
    MULT = mybir.AluOpType.mult
    MAX = mybir.AluOpType.max

    LT = sum(LQs)
    offs = [0, LQs[0]]
    n_ttiles = LT // 128
    NBLK = -(-LT // QB)
    LT_pad = NBLK * QB

    nc = bacc.Bacc("TRN2", target_bir_lowering=False, debug=False,
                   num_devices=NCORES)

    # all big DRAM tensors are host-prearranged so every DMA is one
    # instruction with large per-partition-contiguous descriptors (one
    # dma_start already spreads over all 16 SDMA engines; many small
    # dma_starts just pay ~0.7us DIRECT2D issue cost each on the sync
    # sequencer).
    XT = nc.dram_tensor("XT", [NBLK, 128, n_dblk * QB], BF16,
                        kind="ExternalInput").ap()
    # q/k/v weights batched into one DMA: [128, 3, n_dblk*JW]
    WALL = nc.dram_tensor("WALL", [128, 3 * n_dblk * JW], BF16,
                          kind="ExternalInput").ap()
    WOT = nc.dram_tensor("WOT", [JW, D], BF16, kind="ExternalInput").ap()
    # causal diag-tile masks: CAUS[:, 384-off : 384-off+qbw], off = kt0-q0
    CAUS = nc.dram_tensor("CAUS", [128, 896], BF16, kind="ExternalInput").ap()
    # IND[j, h] = 1 if j // DH == h ; INDT is its transpose
    IND = nc.dram_tensor("IND", [JW, NH], BF16, kind="ExternalInput").ap()
    INDT = nc.dram_tensor("INDT", [NH, JW], BF16, kind="ExternalInput").ap()
    # transposed output, per-q-block contiguous; host reassembles
    n_qblocks = sum(-(-lq // QB) for lq in LQs)
    OUTT = nc.dram_tensor("OUTT", [n_qblocks, 128, n_dblk * QB], F16,
                          kind="ExternalOutput").ap()

    EPS = 1e-12

    def col_blocks(width, bw=QB):
        blocks = []
        c = 0
        while c < width:
            w = min(bw, width - c)
            blocks.append((c, w))
            c += w
        return blocks

    with tile.TileContext(nc) as tc:
        with (
            tc.tile_pool(name="consts", bufs=1) as consts,
            tc.tile_pool(name="proj", bufs=1) as projp,
            tc.tile_pool(name="work", bufs=3) as work,
            tc.tile_pool(name="outp", bufs=2) as outp,
            tc.tile_pool(name="ps_mm", bufs=1, space="PSUM") as ps_mm,
            tc.tile_pool(name="ps_ctxp", bufs=1, space="PSUM") as ps_ctxp,
            tc.tile_pool(name="ps_scp", bufs=6, space="PSUM") as ps_scp,
        ):
            # ---- weights first (first projection matmul needs them); the
            # sync sequencer issues [wq, xt0, wkv, xt1..] only, in that
            # order: the first Q-projection matmul needs just Wq + xt block
            # 0, so those two transfers go ahead of everything else and the
            # small consts use the scalar HWDGE queue.
            wall = consts.tile([128, 3, n_dblk, JW], BF16)
            xtp = tc.alloc_tile_pool(name="xt", bufs=1)
            xt = xtp.tile([128, NBLK, n_dblk, QB], BF16)

            # X^T block-major: each DMA is 1MB of 8KB-contiguous runs. The
            # first Q-projection matmul needs only Wq d-chunks 0-3 and xt0
            # d-chunks 0-1, so those ship first in small transfers; the PE
            # start is data-gated, and this trims the gate to ~380KB.
            wq_half = n_dblk // 2 * JW
            wqv = WALL[:, :n_dblk * JW].rearrange("p (k j) -> p k j",
                                                  k=n_dblk)
            src0 = XT[0].rearrange("p (k w) -> p k w", k=n_dblk)
            half = n_dblk // 2
            nc.sync.dma_start(out=wall[:, 0, :half], in_=wqv[:, :half])
            nc.sync.dma_start(out=xt[:, 0, :2], in_=src0[:, :2])
            nc.sync.dma_start(out=wall[:, 0, half:], in_=wqv[:, half:])
            nc.sync.dma_start(out=xt[:, 0, 2:], in_=src0[:, 2:])
            nc.sync.dma_start(
                out=wall[:, 1:],
                in_=WALL[:, n_dblk * JW:].rearrange("p (s k j) -> p s k j",
                                                    s=2, k=n_dblk))
            for b in range(1, NBLK):
                nc.sync.dma_start(
                    out=xt[:, b],
                    in_=XT[b].rearrange("p (k w) -> p k w", k=n_dblk))
            wqt, wkt, wvt = wall[:, 0], wall[:, 1], wall[:, 2]

            # ---- remaining constants (scalar HWDGE queue) ------------------
            # caus/wot allocs here, but their DMAs are deferred (via the
            # scheduler's wait_until) well past startup: their 0.5MB would
            # otherwise compete with the startup-critical Wq/xt0 delivery on
            # the shared SDMA engines, and their consumers (relu masks,
            # out-projection) run tens of us later.
            caus = consts.tile([128, 896], BF16)
            wot = consts.tile([JW, D], BF16)
            ind = consts.tile([JW, NH], BF16)
            nc.scalar.dma_start(out=ind, in_=IND[:, :])
            indt = consts.tile([NH, JW], BF16)
            nc.scalar.dma_start(out=indt, in_=INDT[:, :])
            eps128 = consts.tile([128, 1], F32)
            nc.vector.memset(eps128, EPS)

            # ---- PE warmup spin --------------------------------------------
            # the HAM clock gate holds the PE at 1.2GHz until it has seen
            # ~3.4us of sustained matmul activity. The first real matmul is
            # DMA-gated to ~10us, so spin ~3us of junk matmuls on a zeroed
            # tile during the DMA wait: the gate flips to 2.4GHz right as the
            # projections start instead of ~4us into them.
            junk = consts.tile([128, 128], BF16)
            nc.vector.memset(junk, 0.0)
            warm_ps = ps_mm.tile([128, 128], F32, tag="mm", name="warm_ps")
            for _ in range(28):
                nc.tensor.matmul(warm_ps[:, :], junk[:, :], junk[:, :],
                                 start=True, stop=True, skip_group_check=True)

            # ---- projections ------------------------------------------------
            # qt/kt in bf16; squares for the norms are taken from the f32
            # PSUM so the scales stay exact for the rounded Q/K. V is
            # projected directly in transposed [tokens, dims] layout (vn).
            qt = projp.tile([JW, LT], BF16)
            kt_ = projp.tile([JW, LT], BF16)
            qsq = projp.tile([JW, LT], BF16)
            ksq = projp.tile([JW, LT], BF16)
            ksc = projp.tile([128, n_ttiles, NH], F32)
            ksc_bf = projp.tile([128, n_ttiles, NH], BF16)
            vn = projp.tile([128, n_ttiles, JW], BF16)

            # qt *= 1/|q| (scores are linear in q; qsb partition p carries
            # head(p)'s scale, matching qt's layout). The chain for block g
            # (sqrt on scalar, reciprocal+multiply on vector) overlaps the
            # projection matmuls of block g+1, so the PE never waits on it:
            # the qsum matmul is emitted with proj(g), the rest one block
            # later via flush_norm.
            norm_pend = []

            def flush_norm():
                c0, w, ssq = norm_pend.pop(0)
                ps_qsb = ps_ctxp.tile([128, QB], F32, tag="ctx_ps",
                                      name="ps_qsb")
                nc.tensor.matmul(ps_qsb[:, :w], indt[:, :], ssq[:, :w],
                                 start=True, stop=True)
                qsb = work.tile([128, QB], F32, tag="qsb")
                nc.vector.reciprocal_approx_fast(out=qsb[:, :w],
                                                 in_=ps_qsb[:, :w])
                # SBUF-only multiply on gpsimd: keeps vector's queue clear
                # for the first relus (the consumer, scores(g), is far away)
                nc.gpsimd.tensor_mul(qt[:, c0:c0 + w], qt[:, c0:c0 + w],
                                     qsb[:, :w])

            cp_i = 0
            for bi, (c0, w) in enumerate(col_blocks(LT)):
                for dst, wmat, sq in ((qt, wqt, qsq), (kt_, wkt, ksq)):
                    ps = ps_scp.tile([JW, QB], F32, tag="sc", name="ps_proj")
                    for k in range(n_dblk):
                        nc.tensor.matmul(
                            ps[:, :w], wmat[:, k, :], xt[:, bi, k, :w],
                            start=(k == 0), stop=(k == n_dblk - 1),
                        )
                    if cp_i % 2 == 0:
                        nc.vector.tensor_copy(dst[:, c0:c0 + w], ps[:, :w])
                    else:
                        nc.scalar.activation(out=dst[:, c0:c0 + w],
                                             in_=ps[:, :w], func=AF.Copy)
                    cp_i += 1
                    # gpsimd is otherwise idle and these are SBUF->SBUF
                    nc.gpsimd.tensor_mul(sq[:, c0:c0 + w],
                                         dst[:, c0:c0 + w],
                                         dst[:, c0:c0 + w])
                # V^T directly off the projection: per 128-token tile,
                # accumulate xt_chunk.T @ Wv_chunk into a [tokens, dims]
                # PSUM tile. Replaces the V projection + 20 PE-mode
                # transposes: transpose-mode doesn't count as PE-busy for
                # the HAM clock gate, and the old transpose+scale pass held
                # the PE semi-idle long enough to re-throttle it to 1.2GHz
                # mid-kernel.
                ntt = w // KT
                ps_vt = ps_scp.tile([128, ntt, KT], F32, tag="sc",
                                    name="ps_vt")
                for tt in range(ntt):
                    for k in range(n_dblk):
                        nc.tensor.matmul(
                            ps_vt[:, tt, :],
                            xt[:, bi, k, ts(tt, KT)],
                            wvt[:, k, :],
                            start=(k == 0), stop=(k == n_dblk - 1),
                            skip_group_check=True,
                        )
                gt0 = c0 // KT
                if cp_i % 2 == 0:
                    nc.vector.tensor_copy(vn[:, gt0:gt0 + ntt, :],
                                          ps_vt[:, :, :])
                else:
                    nc.scalar.activation(out=vn[:, gt0:gt0 + ntt, :],
                                         in_=ps_vt[:, :, :], func=AF.Copy)
                cp_i += 1
                ps_ss = ps_mm.tile([NH, QB], F32, tag="mm", name="ps_qsum")
                nc.tensor.matmul(ps_ss[:, :w], ind[:, :], qsq[:, c0:c0 + w],
                                 start=True, stop=True)
                ssq = work.tile([NH, QB], BF16, tag="ssq")
                nc.scalar.activation(out=ssq[:, :w], in_=ps_ss[:, :w],
                                     func=AF.Sqrt, bias=eps128[:NH, :],
                                     scale=1.0)
                norm_pend.append((c0, w, ssq))
                if len(norm_pend) > 1:
                    flush_norm()
                if bi == 0:
                    # deferred issue: keeps these 0.5MB off the SDMA engines
                    # while the startup-critical wq/xt0 transfers stream in
                    with tc.tile_wait_until(0.018):
                        nc.scalar.dma_start(out=caus, in_=CAUS[:, :])
                        nc.scalar.dma_start(out=wot, in_=WOT[:, :])

            # ---- kscale[t, h] = rsqrt(sum_j ksq[j, t] over head h) ----------
            # (batched after the projections: emitting these per proj block
            # was tried and regressed — the chains serialize on the PE's
            # path through the single-buffer PSUM zone recycling)
            ksum_ps = ps_ctxp.tile([128, n_ttiles * NH], F32, tag="ctx_ps",
                                   name="ksum_ps")
            for tt in range(n_ttiles):
                nc.tensor.matmul(ksum_ps[:, tt * NH:(tt + 1) * NH],
                                 ksq[:, ts(tt, 128)], ind[:, :],
                                 start=True, stop=True, skip_group_check=True)
            nc.scalar.activation(out=ksc[:, :, :].rearrange("p a b -> p (a b)"),
                                 in_=ksum_ps[:, :], func=AF.Sqrt,
                                 bias=eps128[:, :], scale=1.0)
            nc.vector.reciprocal_approx_fast(
                out=ksc[:, :, :].rearrange("p a b -> p (a b)"),
                in_=ksc[:, :, :].rearrange("p a b -> p (a b)"))
            # (the last block's 1/|q| flush happens in the driver, after the
            # first block's scores: ps_qsb may only recycle the ksum bank
            # once its reader, the ksc sqrt above, has been emitted)

            # fold 1/|k| into vn in place: per head, one bf16 2x-mode mul
            # over all tiles, the [128, n_ttiles] scale broadcast over the
            # head's 64 columns via a 0-stride dim
            nc.gpsimd.tensor_copy(
                ksc_bf[:, :, :].rearrange("p a b -> p (a b)"),
                ksc[:, :, :].rearrange("p a b -> p (a b)"))
            for h in range(NH):
                kv = ksc_bf[:, :, h:h + 1]
                kv2 = bass.AP(tensor=kv.tensor, offset=kv.offset,
                              ap=[kv.ap[0], kv.ap[1], [0, DH]])
                nc.vector.tensor_mul(vn[:, :, ts(h, DH)],
                                     vn[:, :, ts(h, DH)], kv2)

            # keep xt resident: releasing it here would make the att pool
            # reuse its SBUF zone, serializing attention start behind the
            # last projection matmul. Both fit in SBUF at bf16 sizes.
            max_nkt = max(LQs) // KT
            att_bufs = 3 if max_nkt <= 10 else (2 if max_nkt <= 14 else 1)
            attp = tc.alloc_tile_pool(name="att", bufs=att_bufs)

            # ---- attention blocks, software-pipelined over q-blocks --------
            # batch-interleaved order: alternating small/large-n_kt blocks
            # smooths the relu load on vector/scalar, and the drain pair
            # becomes the two narrow final blocks (smaller store tail)
            blocks = _block_order(LQs)
            ctx_sbs = {
                b: attp.tile([JW, LQs[b]], BF16, tag=f"ctx_{b}", bufs=1,
                             name=f"ctx_sb{b}")
                for b in range(B)
            }

            state = {}

            def emit_scores(blk):
                b, q0, qw = blk
                ob = offs[b]
                lq = LQs[b]
                n_kt = min((q0 + qw + KT - 1) // KT, lq // KT)
                att_sb = attp.tile([128, max_nkt * NH, QB], BF16,
                                   tag="att_sb", name="att_sb")
                offs_ki = []
                diag_i = 0
                relu_i = 0
                for ki in range(n_kt):
                    k0 = ki * KT
                    # columns < off are fully masked by causality; skip them
                    off = max(0, k0 - q0)
                    offs_ki.append(off)
                    w = qw - off
                    diag = k0 > q0 - KT
                    # per-head 1-bank score tiles: with every ps_scp tile
                    # at most one bank, the pool holds 6 rotation slots in
                    # the same 6 banks, doubling the pipeline depth that
                    # scores/outproj allocations wait on
                    for h in range(NH):
                        sc_h = ps_scp.tile([128, QB], F32, tag="sc",
                                           name="sc_ps")
                        nc.tensor.matmul(
                            sc_h[:, off:qw],
                            kt_[ts(h, DH), ob + k0:ob + k0 + KT],
                            qt[ts(h, DH), ob + q0 + off:ob + q0 + qw],
                            start=True, stop=True,
                        )
                        slot = att_sb[:, ki * NH + h, off:qw]
                        if diag:
                            cs = caus[:, 384:384 + w]
                            if (diag_i + h) % 2 == 0:
                                nc.vector.scalar_tensor_tensor(
                                    out=slot, in0=sc_h[:, off:qw],
                                    scalar=0.0, in1=cs, op0=MAX, op1=MULT)
                            else:
                                nc.scalar.activation(
                                    out=slot, in_=sc_h[:, off:qw],
                                    func=AF.Relu)
                                nc.vector.tensor_mul(slot, slot, cs)
                        else:
                            if (relu_i + h) % 2 == 1:
                                nc.scalar.activation(
                                    out=slot, in_=sc_h[:, off:qw],
                                    func=AF.Relu)
                            else:
                                nc.vector.tensor_scalar_max(
                                    out=slot, in0=sc_h[:, off:qw],
                                    scalar1=0.0)
                    if diag:
                        diag_i += 1
                    else:
                        relu_i += 1
                state[blk] = (att_sb, n_kt, offs_ki)

            def emit_ctx(blk_i, blk):
                b, q0, qw = blk
                ob = offs[b]
                ctx_sb = ctx_sbs[b]
                att_sb, n_kt, offs_ki = state.pop(blk)
                # col-tiled pair: both heads accumulate in one PSUM bank.
                # the final block borrows the (idle by then) ps_mm bank so
                # its matmuls don't wait on the previous block's ctx copy.
                if blk_i == n_qblocks - 1:
                    ctx_ps = ps_mm.tile([128, QB], F32, tag="mm",
                                        name="ctx_ps_last")
                else:
                    ctx_ps = ps_ctxp.tile([128, QB], F32, tag="ctx_ps",
                                          name="ctx_ps")
                assert offs_ki[0] == 0  # first tile always starts the bank
                for ki in range(n_kt):
                    gtt = (ob + ki * KT) // KT
                    off = offs_ki[ki]
                    for h in range(NH):
                        nc.tensor.matmul(
                            ctx_ps[ts(h, DH), off:qw],
                            vn[:, gtt, ts(h, DH)],
                            att_sb[:, ki * NH + h, off:qw],
                            start=(ki == 0), stop=(ki == n_kt - 1),
                            tile_position=(0, h * DH),
                            skip_group_check=True,
                        )
                # 1/|q| already folded into the queries; plain copy. Scalar
                # mid-stream (vector carries the relus there); vector for the
                # last two blocks, where it is idle and scalar still has the
                # previous block's out-copies queued.
                if blk_i >= n_qblocks - 2:
                    nc.vector.tensor_copy(ctx_sb[:, q0:q0 + qw],
                                          ctx_ps[:, :qw])
                else:
                    nc.scalar.activation(out=ctx_sb[:, q0:q0 + qw],
                                         in_=ctx_ps[:, :qw], func=AF.Copy)

            def emit_outproj(blk_i, blk):
                b, q0, qw = blk
                ob = offs[b]
                ctx_sb = ctx_sbs[b]
                # output projection (transposed layout), this q-block only;
                # per-chunk 1-bank PSUM tiles (see scores); all 8 dout
                # chunks gather into one SBUF tile and ship in a single DMA.
                o_all = outp.tile([128, n_dblk, QB], F16, tag="o_sb")
                dst = OUTT[blk_i].rearrange("p (g w) -> p g w", g=n_dblk)
                for g in range(n_dblk):
                    ps = ps_scp.tile([128, QB], F32, tag="sc", name="ps_out")
                    nc.tensor.matmul(ps[:, :qw], wot[:, ts(g, 128)],
                                     ctx_sb[:, q0:q0 + qw],
                                     start=True, stop=True)
                    if g % 2 == 0:
                        nc.vector.tensor_copy(o_all[:, g, :qw], ps[:, :qw])
                    else:
                        nc.scalar.activation(out=o_all[:, g, :qw],
                                             in_=ps[:, :qw], func=AF.Copy)
                nc.sync.dma_start(out=dst[:, :, :qw], in_=o_all[:, :, :qw])

            def emit_outproj_pair(i1, blk1, i2, blk2):
                # drain: interleave the two blocks' dout chunks so each
                # block's PSUM->SBUF copies retire under the other block's
                # matmuls instead of stalling the PE via pool rotation
                parts = []
                for blk_i, (b, q0, qw) in ((i1, blk1), (i2, blk2)):
                    o_all = outp.tile([128, n_dblk, QB], F16, tag="o_sb")
                    dst = OUTT[blk_i].rearrange("p (g w) -> p g w", g=n_dblk)
                    parts.append((ctx_sbs[b], q0, qw, o_all, dst))
                for g in range(n_dblk):
                    for pi, (ctx_sb, q0, qw, o_all, dst) in enumerate(parts):
                        ps = ps_scp.tile([128, QB], F32, tag="sc",
                                         name="ps_out")
                        nc.tensor.matmul(ps[:, :qw], wot[:, ts(g, 128)],
                                         ctx_sb[:, q0:q0 + qw],
                                         start=True, stop=True)
                        if (g + pi) % 2 == 0:
                            nc.vector.tensor_copy(o_all[:, g, :qw],
                                                  ps[:, :qw])
                        else:
                            nc.scalar.activation(out=o_all[:, g, :qw],
                                                 in_=ps[:, :qw],
                                                 func=AF.Copy)
                        if g in (2, n_dblk - 3):
                            # ship finished chunks early so the final store
                            # tail after the last matmul stays small
                            lo = 0 if g == 2 else 3
                            nc.sync.dma_start(out=dst[:, lo:g + 1, :qw],
                                              in_=o_all[:, lo:g + 1, :qw])
                g0 = n_dblk - 2
                for ctx_sb, q0, qw, o_all, dst in parts:
                    nc.sync.dma_start(out=dst[:, g0:, :qw],
                                      in_=o_all[:, g0:, :qw])

            def emit_ctx_out(blk_i, blk):
                emit_ctx(blk_i, blk)
                emit_outproj(blk_i, blk)

            # depth-2 software pipeline: scores of blocks i and i+1 are both
            # in flight before ctx of block i-1, giving the relu/copy engines
            # a full extra block of slack before the PE consumes their output.
            emit_scores(blocks[0])
            flush_norm()  # last block's qt scale, behind the first relus
            if len(blocks) > 1:
                emit_scores(blocks[1])
            for i, blk in enumerate(blocks):
                if i <= 1:
                    continue
                emit_scores(blk)
                if i > 1:
                    emit_ctx_out(i - 2, blocks[i - 2])
            # drain: interleave the last two blocks' ctx and out-projection
            # so each out-projection's ctx copy completes under the other
            # block's matmuls instead of stalling the PE
            n = len(blocks)
            if n > 1:
                emit_ctx(n - 2, blocks[-2])
                emit_ctx(n - 1, blocks[-1])
                emit_outproj_pair(n - 2, blocks[-2], n - 1, blocks[-1])
            else:
                emit_ctx_out(0, blocks[0])
            attp.release()
            xtp.release()

    nc.compile()
    return nc


def _prepare(X, masks, Wq, Wk, Wv, Wo):
    import ml_dtypes
    BF = ml_dtypes.bfloat16
    F8 = ml_dtypes.float8_e4m3

    X = np.asarray(X, dtype=np.float32)
    masks = np.asarray(masks)
    Wq = np.asarray(Wq, dtype=np.float32)
    Wk = np.asarray(Wk, dtype=np.float32)
    Wv = np.asarray(Wv, dtype=np.float32)
    Wo = np.asarray(Wo, dtype=np.float32)

    idxs = [np.where(masks[b] != 0)[0] for b in range(B)]
    # 256-multiples so fp8 DoubleRow k-tile pairs align for both batches
    LQs = [max(256, int(-(-len(ix) // 256) * 256)) for ix in idxs]
    LT = sum(LQs)
    offs = [0, LQs[0]]
    QBK = 512
    NBLK = -(-LT // QBK)
    LT_pad = NBLK * QBK
    n_dblk = D // 128

    # compacted, transposed X: columns = valid tokens (zero-padded)
    XTc = np.zeros((D, LT_pad), dtype=np.float32)
    for b in range(B):
        XTc[:, offs[b]:offs[b] + len(idxs[b])] = X[b].T[:, idxs[b]]
    # DMA-friendly: [NBLK, 128, n_dblk*QBK], per-partition contiguous
    XTa = np.ascontiguousarray(
        XTc.reshape(n_dblk, 128, NBLK, QBK).transpose(2, 1, 0, 3)
        .reshape(NBLK, 128, n_dblk * QBK)).astype(BF)

    caus = (np.arange(896)[None, :] - 384 >= np.arange(128)[:, None])

    nc = _build_program(LQs)

    def warr(wT):  # [D, JW] -> [128, n_dblk*JW] per-partition contiguous
        return np.ascontiguousarray(
            wT.reshape(n_dblk, 128, JW).transpose(1, 0, 2)
            .reshape(128, n_dblk * JW)).astype(BF)

    in_maps = []
    for c in range(NCORES):
        jsl = slice(c * JW, (c + 1) * JW)
        ind = np.zeros((JW, NH), dtype=np.float32)
        for h in range(NH):
            ind[h * DH:(h + 1) * DH, h] = 1.0
        in_maps.append({
            "XT": XTa,
            "WALL": np.ascontiguousarray(np.concatenate(
                [warr(Wq[jsl, :].T), warr(Wk[jsl, :].T),
                 warr(Wv[jsl, :].T)], axis=1)),
            "WOT": np.ascontiguousarray(Wo[:, jsl].T).astype(BF),
            "CAUS": caus.astype(BF),
            "IND": ind.astype(BF),
            "INDT": np.ascontiguousarray(ind.T).astype(BF),
        })

    return nc, in_maps, (idxs, LQs, LT, offs)


def _unshard(results, meta):
    idxs, LQs, LT, offs = meta
    n_dblk = D // 128
    blocks = _block_order(LQs)  # must match the device emission order

    partial = np.zeros((D, LT), dtype=np.float64)
    for c in range(NCORES):
        # OUTT[i, p, g*QB + w] = out[g*128 + p, ob + q0 + w] for block i
        ot = results[c]["OUTT"].astype(np.float64).reshape(
            len(blocks), 128, n_dblk, QB)
        for i, (b, q0, qw) in enumerate(blocks):
            cols = slice(offs[b] + q0, offs[b] + q0 + qw)
            partial[:, cols] += ot[i, :, :, :qw].transpose(1, 0, 2).reshape(
                D, qw)
    partial = partial.T  # [LT, D]

    out = np.zeros((B, S, D), dtype=np.float32)
    for b in range(B):
        out[b, idxs[b], :] = partial[offs[b]:offs[b] + len(idxs[b]), :].astype(
            np.float32)
    return out


def kernel(X, masks, Wq, Wk, Wv, Wo):
    from concourse.bass_utils import run_bass_kernel_spmd

    nc, in_maps, meta = _prepare(X, masks, Wq, Wk, Wv, Wo)
    res = run_bass_kernel_spmd(nc, in_maps, list(range(NCORES)))
    return _unshard(res.results, meta)


def profile_run(inputs, tmpdir=None):
    """Used by test.py: same program, run with NTFF tracing enabled."""
    from concourse.bass_utils import run_bass_kernel_spmd

    nc, in_maps, meta = _prepare(**inputs)
    res = run_bass_kernel_spmd(nc, in_maps, list(range(NCORES)), trace=True,
                               tmpdir=tmpdir)
    res.output = _unshard(res.results, meta)
    return res



# revision 26
# speedup vs baseline: 1.3204x; 1.0915x over previous
"""Sparse (relu-cosine, causal+padding-masked) attention on 8 TRN2 NeuronCores.

Contract: kernel(**inputs) takes the full unsharded inputs and returns the
full [B, S, D] output. Internally:
  - host: compact each batch's tokens to the mask-valid ones (queries and
    keys share the same validity mask, so causal structure stays exactly
    lower-triangular in compacted space and all masking disappears),
    transpose X, slice per-head-pair weights, pad to tile multiples.
  - device (SPMD, 8 cores, 2 heads per core): Q/K projections plus a
    direct transposed V projection (per 128-token tile, xt_chunk.T @ Wv),
    cosine normalization folded into V (1/||k||, applied in the PSUM->SBUF
    copy) and the query vectors (1/||q||), relu(QK^T) with triangular
    masks only on diagonal tiles, context accumulation (col-tiled pairs),
    and a partial output projection (transposed layout) through this
    core's 128 columns of Wo.
  - host: sum the 8 partial outputs, scatter rows back to the full
    [B, S, D] layout (masked query rows are exactly zero).

Matmul operands are bf16; every accumulation (PSUM) is fp32 and the
norm scales (1/||q||, 1/||k||) are computed from the fp32 sums.
Attention is software-pipelined: scores of blocks i..i+2 are issued to
the PE before the context matmuls of block i-1, q-blocks alternate
between the batches, and each key tile's two heads share one 2-bank
PSUM tile so a single relu op covers the pair.

Perf notes (measured on HW): the PE's HAM clock gate runs the array at
1.2GHz until it sees ~3.4us of sustained matmuls (and PE-mode
transposes don't count as busy!) - hence the warmup spin and the
no-transpose V^T projection; with those, the kernel holds 2.4GHz.
All large DMAs are single instructions with >=1KB per-partition
contiguous descriptors (host pre-arranges the DRAM layouts); every
extra dma_start costs ~0.7us of serial DIRECT2D issue. caus/wot loads
are deferred so the startup-critical wq/xt0 transfers own the SDMA
engines. reciprocal_approx_fast is ~5x faster than InstReciprocal.
fp8e4m3 (DoubleRow) was tried and reverted: each quantized stage
(att/vn/ctx/wot) alone adds ~2.3-3.3%% fro error vs the 2%% budget,
and quantized Q/K directions give the same error class.
"""

import numpy as np

B, S, D, H = 2, 2048, 1024, 16
DH = D // H
NCORES = 8
HEADS_PER_CORE = H // NCORES  # 2
NH = HEADS_PER_CORE
JW = HEADS_PER_CORE * DH  # 128, per-core head-dim slice width
QB = 512  # query block width (one fp32 PSUM bank)
KT = 128  # key tile (partition dim)


def _col_blocks(width, bw=QB):
    blocks = []
    c = 0
    while c < width:
        w = min(bw, width - c)
        blocks.append((c, w))
        c += w
    return blocks


def _block_order(LQs):
    """Batch-interleaved q-block order (b0B0, b1B0, b0B1, ...)."""
    per = [_col_blocks(lq) for lq in LQs]
    out = []
    for j in range(max(len(p) for p in per)):
        for b in range(len(LQs)):
            if j < len(per[b]):
                out.append((b, per[b][j][0], per[b][j][1]))
    return out


def _build_program(LQs, n_dblk=D // 128):
    import concourse.bass as bass
    import concourse.mybir as mybir
    import concourse.tile as tile
    from concourse import bacc
    from concourse.bass import ts

    F32 = mybir.dt.float32
    F32R = mybir.dt.float32r
    F16 = mybir.dt.float16
    BF16 = mybir.dt.bfloat16
    F8E4 = mybir.dt.float8e4
    DROW = mybir.MatmulPerfMode.DoubleRow
    AF = mybir.ActivationFunctionType
    MULT = mybir.AluOpType.mult
    MAX = mybir.AluOpType.max

    LT = sum(LQs)
    offs = [0, LQs[0]]
    n_ttiles = LT // 128
    NBLK = -(-LT // QB)
    LT_pad = NBLK * QB

    nc = bacc.Bacc("TRN2", target_bir_lowering=False, debug=False,
                   num_devices=NCORES)

    # all big DRAM tensors are host-prearranged so every DMA is one
    # instruction with large per-partition-contiguous descriptors (one
    # dma_start already spreads over all 16 SDMA engines; many small
    # dma_starts just pay ~0.7us DIRECT2D issue cost each on the sync
    # sequencer).
    XT = nc.dram_tensor("XT", [NBLK, 128, n_dblk * QB], BF16,
                        kind="ExternalInput").ap()
    # q/k/v weights batched into one DMA: [128, 3, n_dblk*JW]
    WALL = nc.dram_tensor("WALL", [128, 3 * n_dblk * JW], BF16,
                          kind="ExternalInput").ap()
    WOT = nc.dram_tensor("WOT", [JW, D], BF16, kind="ExternalInput").ap()
    # causal diag-tile masks: CAUS[:, 384-off : 384-off+qbw], off = kt0-q0
    CAUS = nc.dram_tensor("CAUS", [128, 896], BF16, kind="ExternalInput").ap()
    # IND[j, h] = 1 if j // DH == h ; INDT is its transpose
    IND = nc.dram_tensor("IND", [JW, NH], BF16, kind="ExternalInput").ap()
    INDT = nc.dram_tensor("INDT", [NH, JW], BF16, kind="ExternalInput").ap()
    # transposed output, per-q-block contiguous; host reassembles
    n_qblocks = sum(-(-lq // QB) for lq in LQs)
    OUTT = nc.dram_tensor("OUTT", [n_qblocks, 128, n_dblk * QB], F16,
                          kind="ExternalOutput").ap()

    EPS = 1e-12

    def col_blocks(width, bw=QB):
        blocks = []
        c = 0
        while c < width:
            w = min(bw, width - c)
            blocks.append((c, w))
            c += w
        return blocks

    with tile.TileContext(nc) as tc:
        with (
            tc.tile_pool(name="consts", bufs=1) as consts,
            tc.tile_pool(name="proj", bufs=1) as projp,
            tc.tile_pool(name="work", bufs=3) as work,
            tc.tile_pool(name="outp", bufs=2) as outp,
            tc.tile_pool(name="ps_mm", bufs=1, space="PSUM") as ps_mm,
            tc.tile_pool(name="ps_ctxp", bufs=1, space="PSUM") as ps_ctxp,
            tc.tile_pool(name="ps_scp", bufs=3, space="PSUM") as ps_scp,
        ):
            # ---- weights first (first projection matmul needs them); the
            # sync sequencer issues [wq, xt0, wkv, xt1..] only, in that
            # order: the first Q-projection matmul needs just Wq + xt block
            # 0, so those two transfers go ahead of everything else and the
            # small consts use the scalar HWDGE queue.
            wall = consts.tile([128, 3, n_dblk, JW], BF16)
            xtp = tc.alloc_tile_pool(name="xt", bufs=1)
            xt = xtp.tile([128, NBLK, n_dblk, QB], BF16)

            # X^T block-major: each DMA is 1MB of 8KB-contiguous runs. The
            # first Q-projection matmul needs only Wq d-chunks 0-3 and xt0
            # d-chunks 0-1, so those ship first in small transfers; the PE
            # start is data-gated, and this trims the gate to ~380KB.
            wq_half = n_dblk // 2 * JW
            wqv = WALL[:, :n_dblk * JW].rearrange("p (k j) -> p k j",
                                                  k=n_dblk)
            src0 = XT[0].rearrange("p (k w) -> p k w", k=n_dblk)
            half = n_dblk // 2
            nc.sync.dma_start(out=wall[:, 0, :half], in_=wqv[:, :half])
            nc.sync.dma_start(out=xt[:, 0, :2], in_=src0[:, :2])
            nc.sync.dma_start(out=wall[:, 0, half:], in_=wqv[:, half:])
            nc.sync.dma_start(out=xt[:, 0, 2:], in_=src0[:, 2:])
            nc.sync.dma_start(
                out=wall[:, 1:],
                in_=WALL[:, n_dblk * JW:].rearrange("p (s k j) -> p s k j",
                                                    s=2, k=n_dblk))
            for b in range(1, NBLK):
                nc.sync.dma_start(
                    out=xt[:, b],
                    in_=XT[b].rearrange("p (k w) -> p k w", k=n_dblk))
            wqt, wkt, wvt = wall[:, 0], wall[:, 1], wall[:, 2]

            # ---- remaining constants (scalar HWDGE queue) ------------------
            # caus/wot allocs here, but their DMAs are deferred (via the
            # scheduler's wait_until) well past startup: their 0.5MB would
            # otherwise compete with the startup-critical Wq/xt0 delivery on
            # the shared SDMA engines, and their consumers (relu masks,
            # out-projection) run tens of us later.
            caus = consts.tile([128, 896], BF16)
            wot = consts.tile([JW, D], BF16)
            ind = consts.tile([JW, NH], BF16)
            nc.scalar.dma_start(out=ind, in_=IND[:, :])
            indt = consts.tile([NH, JW], BF16)
            nc.scalar.dma_start(out=indt, in_=INDT[:, :])
            eps128 = consts.tile([128, 1], F32)
            nc.vector.memset(eps128, EPS)

            # ---- PE warmup spin --------------------------------------------
            # the HAM clock gate holds the PE at 1.2GHz until it has seen
            # ~3.4us of sustained matmul activity. The first real matmul is
            # DMA-gated to ~10us, so spin ~3us of junk matmuls on a zeroed
            # tile during the DMA wait: the gate flips to 2.4GHz right as the
            # projections start instead of ~4us into them.
            junk = consts.tile([128, 128], BF16)
            nc.vector.memset(junk, 0.0)
            warm_ps = ps_mm.tile([128, 128], F32, tag="mm", name="warm_ps")
            for _ in range(28):
                nc.tensor.matmul(warm_ps[:, :], junk[:, :], junk[:, :],
                                 start=True, stop=True, skip_group_check=True)

            # ---- projections ------------------------------------------------
            # qt/kt in bf16; squares for the norms are taken from the f32
            # PSUM so the scales stay exact for the rounded Q/K. V is
            # projected directly in transposed [tokens, dims] layout (vn).
            qt = projp.tile([JW, LT], BF16)
            kt_ = projp.tile([JW, LT], BF16)
            qsq = projp.tile([JW, LT], BF16)
            ksq = projp.tile([JW, LT], BF16)
            ksc = projp.tile([128, n_ttiles, NH], F32)
            vn = projp.tile([128, n_ttiles, JW], BF16)
            # per-token |k|^2 sums for the whole run live in one bank; the
            # per-block matmuls write disjoint column slices
            ksum_all = ps_ctxp.tile([128, n_ttiles, NH], F32, tag="ctx_ps",
                                    name="ksum_ps")

            # qt *= 1/|q| (scores are linear in q; qsb partition p carries
            # head(p)'s scale, matching qt's layout). The chain for block g
            # (sqrt on scalar, reciprocal+multiply on vector) overlaps the
            # projection matmuls of block g+1, so the PE never waits on it:
            # the qsum matmul is emitted with proj(g), the rest one block
            # later via flush_norm.
            norm_pend = []

            def flush_norm():
                c0, w, ssq = norm_pend.pop(0)
                ps_qsb = ps_mm.tile([128, QB], F32, tag="mm", name="ps_qsb")
                nc.tensor.matmul(ps_qsb[:, :w], indt[:, :], ssq[:, :w],
                                 start=True, stop=True)
                qsb = work.tile([128, QB], F32, tag="qsb")
                nc.vector.reciprocal_approx_fast(out=qsb[:, :w],
                                                 in_=ps_qsb[:, :w])
                # SBUF-only multiply on gpsimd: keeps vector's queue clear
                # for the first relus (the consumer, scores(g), is far away)
                nc.gpsimd.tensor_mul(qt[:, c0:c0 + w], qt[:, c0:c0 + w],
                                     qsb[:, :w])

            # V^T tiles are normalized one block late: the copy out of PSUM
            # is a single vector multiply by 1/|k| (no separate scale pass),
            # and the lag gives the ksq -> ksum -> sqrt -> recip chain a full
            # projection block (~6us) to complete off the critical path.
            vt_pend = []

            def flush_vt():
                gt0, ntt, ps_vt = vt_pend.pop(0)
                for h in range(NH):
                    ksl = ksc[:, gt0:gt0 + ntt, h:h + 1]
                    k3 = bass.AP(tensor=ksl.tensor, offset=ksl.offset,
                                 ap=[ksl.ap[0], ksl.ap[1], [0, DH]])
                    nc.vector.tensor_mul(vn[:, gt0:gt0 + ntt, ts(h, DH)],
                                         ps_vt[:, :, ts(h, DH)], k3)

            cp_i = 0
            for bi, (c0, w) in enumerate(col_blocks(LT)):
                # Q and K share one 2-bank PSUM slot (the pool holds 3
                # 2-bank slots; per block this leaves room for this block's
                # V^T tile plus the previous block's still-pending one)
                ps_qk = ps_scp.tile([JW, 2, QB], F32, tag="sc", name="ps_qk")
                for j, (dst, wmat, sq) in enumerate(((qt, wqt, qsq),
                                                     (kt_, wkt, ksq))):
                    for k in range(n_dblk):
                        nc.tensor.matmul(
                            ps_qk[:, j, :w], wmat[:, k, :], xt[:, bi, k, :w],
                            start=(k == 0), stop=(k == n_dblk - 1),
                            skip_group_check=True,
                        )
                    if cp_i % 2 == 0:
                        nc.vector.tensor_copy(dst[:, c0:c0 + w],
                                              ps_qk[:, j, :w])
                    else:
                        nc.scalar.activation(out=dst[:, c0:c0 + w],
                                             in_=ps_qk[:, j, :w],
                                             func=AF.Copy)
                    cp_i += 1
                    # gpsimd is otherwise idle and these are SBUF->SBUF
                    nc.gpsimd.tensor_mul(sq[:, c0:c0 + w],
                                         dst[:, c0:c0 + w],
                                         dst[:, c0:c0 + w])
                # previous block's deferred norm flushes go before this
                # block's V^T matmuls so their PSUM slot frees in time
                if len(norm_pend) > 0 and bi > 0:
                    flush_norm()
                if len(vt_pend) > 0:
                    flush_vt()
                # V^T directly off the projection: per 128-token tile,
                # accumulate xt_chunk.T @ Wv_chunk into a [tokens, dims]
                # PSUM tile. Replaces the V projection + 20 PE-mode
                # transposes: transpose-mode doesn't count as PE-busy for
                # the HAM clock gate, and the old transpose+scale pass held
                # the PE semi-idle long enough to re-throttle it to 1.2GHz
                # mid-kernel.
                ntt = w // KT
                gt0 = c0 // KT
                ps_vt = ps_scp.tile([128, ntt, KT], F32, tag="sc",
                                    name="ps_vt")
                for tt in range(ntt):
                    for k in range(n_dblk):
                        nc.tensor.matmul(
                            ps_vt[:, tt, :],
                            xt[:, bi, k, ts(tt, KT)],
                            wvt[:, k, :],
                            start=(k == 0), stop=(k == n_dblk - 1),
                            skip_group_check=True,
                        )
                # this block's kscale chain: ksum matmuls into the shared
                # bank, then sqrt+recip on the slice
                for tt in range(ntt):
                    nc.tensor.matmul(ksum_all[:, gt0 + tt, :],
                                     ksq[:, c0 + tt * KT:c0 + (tt + 1) * KT],
                                     ind[:, :], start=True, stop=True,
                                     skip_group_check=True)
                ksl = ksc[:, gt0:gt0 + ntt, :].rearrange("p a b -> p (a b)")
                nc.scalar.activation(
                    out=ksl,
                    in_=ksum_all[:, gt0:gt0 + ntt, :].rearrange(
                        "p a b -> p (a b)"),
                    func=AF.Sqrt, bias=eps128[:, :], scale=1.0)
                nc.vector.reciprocal_approx_fast(out=ksl, in_=ksl)
                ps_ss = ps_mm.tile([NH, QB], F32, tag="mm", name="ps_qsum")
                nc.tensor.matmul(ps_ss[:, :w], ind[:, :], qsq[:, c0:c0 + w],
                                 start=True, stop=True)
                ssq = work.tile([NH, QB], BF16, tag="ssq")
                nc.scalar.activation(out=ssq[:, :w], in_=ps_ss[:, :w],
                                     func=AF.Sqrt, bias=eps128[:NH, :],
                                     scale=1.0)
                norm_pend.append((c0, w, ssq))
                vt_pend.append((gt0, ntt, ps_vt))
                if bi == 0:
                    # deferred issue: keeps these 0.5MB off the SDMA engines
                    # while the startup-critical wq/xt0 transfers stream in
                    with tc.tile_wait_until(0.018):
                        nc.scalar.dma_start(out=caus, in_=CAUS[:, :])
                        nc.scalar.dma_start(out=wot, in_=WOT[:, :])

            flush_vt()  # last block's V^T normalize+copy

            # keep xt resident: releasing it here would make the att pool
            # reuse its SBUF zone, serializing attention start behind the
            # last projection matmul. Both fit in SBUF at bf16 sizes.
            max_nkt = max(LQs) // KT
            att_bufs = 4 if max_nkt <= 10 else (2 if max_nkt <= 14 else 1)
            attp = tc.alloc_tile_pool(name="att", bufs=att_bufs)

            # ---- attention blocks, software-pipelined over q-blocks --------
            # batch-interleaved order: alternating small/large-n_kt blocks
            # smooths the relu load on vector/scalar, and the drain pair
            # becomes the two narrow final blocks (smaller store tail)
            blocks = _block_order(LQs)
            ctx_sbs = {
                b: attp.tile([JW, LQs[b]], BF16, tag=f"ctx_{b}", bufs=1,
                             name=f"ctx_sb{b}")
                for b in range(B)
            }

            state = {}

            def emit_scores(blk):
                b, q0, qw = blk
                ob = offs[b]
                lq = LQs[b]
                n_kt = min((q0 + qw + KT - 1) // KT, lq // KT)
                att_sb = attp.tile([128, max_nkt * NH, QB], BF16,
                                   tag="att_sb", name="att_sb")
                offs_ki = []
                for ki in range(n_kt):
                    k0 = ki * KT
                    # columns < off are fully masked by causality; skip them
                    off = max(0, k0 - q0)
                    offs_ki.append(off)
                    w = qw - off
                    diag = k0 > q0 - KT
                    # both heads' score tiles in one 2-bank PSUM tile: the
                    # matmuls still run concurrently (disjoint row groups),
                    # and ONE relu op covers the pair - halving the relu op
                    # count and fixed overhead on vector/scalar
                    sc = ps_scp.tile([128, NH, QB], F32, tag="sc",
                                     name="sc_ps")
                    for h in range(NH):
                        nc.tensor.matmul(
                            sc[:, h, off:qw],
                            kt_[ts(h, DH), ob + k0:ob + k0 + KT],
                            qt[ts(h, DH), ob + q0 + off:ob + q0 + qw],
                            start=True, stop=True, skip_group_check=True,
                        )
                    slot = att_sb[:, ki * NH:(ki + 1) * NH, off:qw]
                    src = sc[:, :, off:qw]
                    if diag:
                        cs = caus[:, 384:384 + w]
                        cs2 = bass.AP(tensor=cs.tensor, offset=cs.offset,
                                      ap=[cs.ap[0], [0, NH], cs.ap[1]])
                        if ki % 2 == 0:
                            nc.vector.scalar_tensor_tensor(
                                out=slot, in0=src, scalar=0.0, in1=cs2,
                                op0=MAX, op1=MULT)
                        else:
                            nc.scalar.activation(out=slot, in_=src,
                                                 func=AF.Relu)
                            nc.vector.tensor_mul(slot, slot, cs2)
                    else:
                        if ki % 2 == 1:
                            nc.scalar.activation(out=slot, in_=src,
                                                 func=AF.Relu)
                        else:
                            nc.vector.tensor_scalar_max(
                                out=slot, in0=src, scalar1=0.0)
                state[blk] = (att_sb, n_kt, offs_ki)

            def emit_ctx(blk_i, blk):
                b, q0, qw = blk
                ob = offs[b]
                ctx_sb = ctx_sbs[b]
                att_sb, n_kt, offs_ki = state.pop(blk)
                # col-tiled pair: both heads accumulate in one PSUM bank.
                # the final block borrows the (idle by then) ps_mm bank so
                # its matmuls don't wait on the previous block's ctx copy.
                if blk_i == n_qblocks - 1:
                    ctx_ps = ps_mm.tile([128, QB], F32, tag="mm",
                                        name="ctx_ps_last")
                else:
                    ctx_ps = ps_ctxp.tile([128, QB], F32, tag="ctx_ps",
                                          name="ctx_ps")
                assert offs_ki[0] == 0  # first tile always starts the bank
                for ki in range(n_kt):
                    gtt = (ob + ki * KT) // KT
                    off = offs_ki[ki]
                    for h in range(NH):
                        nc.tensor.matmul(
                            ctx_ps[ts(h, DH), off:qw],
                            vn[:, gtt, ts(h, DH)],
                            att_sb[:, ki * NH + h, off:qw],
                            start=(ki == 0), stop=(ki == n_kt - 1),
                            tile_position=(0, h * DH),
                            skip_group_check=True,
                        )
                # 1/|q| already folded into the queries; plain copy. Scalar
                # mid-stream (vector carries the relus there); vector for the
                # last two blocks, where it is idle and scalar still has the
                # previous block's out-copies queued.
                if blk_i >= n_qblocks - 2:
                    nc.vector.tensor_copy(ctx_sb[:, q0:q0 + qw],
                                          ctx_ps[:, :qw])
                else:
                    nc.scalar.activation(out=ctx_sb[:, q0:q0 + qw],
                                         in_=ctx_ps[:, :qw], func=AF.Copy)

            def emit_outproj(blk_i, blk):
                b, q0, qw = blk
                ob = offs[b]
                ctx_sb = ctx_sbs[b]
                # output projection (transposed layout), this q-block only;
                # chunk PAIRS share a 2-bank PSUM tile so one copy op moves
                # both; all 8 dout chunks gather into one SBUF tile and
                # ship in a single DMA.
                o_all = outp.tile([128, n_dblk, QB], F16, tag="o_sb")
                dst = OUTT[blk_i].rearrange("p (g w) -> p g w", g=n_dblk)
                for gp in range(n_dblk // 2):
                    ps = ps_scp.tile([128, 2, QB], F32, tag="sc",
                                     name="ps_out")
                    for j in range(2):
                        nc.tensor.matmul(ps[:, j, :qw],
                                         wot[:, ts(2 * gp + j, 128)],
                                         ctx_sb[:, q0:q0 + qw],
                                         start=True, stop=True,
                                         skip_group_check=True)
                    g2 = 2 * gp
                    if gp % 2 == 0:
                        nc.vector.tensor_copy(o_all[:, g2:g2 + 2, :qw],
                                              ps[:, :, :qw])
                    else:
                        nc.scalar.activation(out=o_all[:, g2:g2 + 2, :qw],
                                             in_=ps[:, :, :qw], func=AF.Copy)
                nc.sync.dma_start(out=dst[:, :, :qw], in_=o_all[:, :, :qw])

            def emit_outproj_pair(i1, blk1, i2, blk2):
                # drain: interleave the two blocks' dout chunks so each
                # block's PSUM->SBUF copies retire under the other block's
                # matmuls instead of stalling the PE via pool rotation
                parts = []
                for blk_i, (b, q0, qw) in ((i1, blk1), (i2, blk2)):
                    o_all = outp.tile([128, n_dblk, QB], F16, tag="o_sb")
                    dst = OUTT[blk_i].rearrange("p (g w) -> p g w", g=n_dblk)
                    parts.append((ctx_sbs[b], q0, qw, o_all, dst))
                for gp in range(n_dblk // 2):
                    for pi, (ctx_sb, q0, qw, o_all, dst) in enumerate(parts):
                        ps = ps_scp.tile([128, 2, QB], F32, tag="sc",
                                         name="ps_out")
                        for j in range(2):
                            nc.tensor.matmul(ps[:, j, :qw],
                                             wot[:, ts(2 * gp + j, 128)],
                                             ctx_sb[:, q0:q0 + qw],
                                             start=True, stop=True,
                                             skip_group_check=True)
                        g2 = 2 * gp
                        if (gp + pi) % 2 == 0:
                            nc.vector.tensor_copy(o_all[:, g2:g2 + 2, :qw],
                                                  ps[:, :, :qw])
                        else:
                            nc.scalar.activation(
                                out=o_all[:, g2:g2 + 2, :qw],
                                in_=ps[:, :, :qw], func=AF.Copy)
                        # ship finished chunks early so the final store
                        # tail after the last matmul stays small
                        if gp == 1:
                            nc.sync.dma_start(out=dst[:, :4, :qw],
                                              in_=o_all[:, :4, :qw])
                        elif gp == 2:
                            nc.sync.dma_start(out=dst[:, 4:6, :qw],
                                              in_=o_all[:, 4:6, :qw])
                for ctx_sb, q0, qw, o_all, dst in parts:
                    nc.sync.dma_start(out=dst[:, 6:, :qw],
                                      in_=o_all[:, 6:, :qw])

            def emit_ctx_out(blk_i, blk):
                emit_ctx(blk_i, blk)
                emit_outproj(blk_i, blk)

            # depth-3 software pipeline: scores of blocks i, i+1, i+2 are all
            # in flight before ctx of block i-1, giving the relu/copy engines
            # two extra blocks of slack before the PE consumes their output
            # (the pipeline-fill window is where the HAM clock gate used to
            # re-throttle: the PE needs queued score work while the first
            # relus drain).
            n = len(blocks)
            emit_scores(blocks[0])
            flush_norm()  # last block's qt scale, behind the first relus
            if n > 1:
                emit_scores(blocks[1])
            if n > 2:
                emit_scores(blocks[2])
            for i in range(3, n):
                emit_scores(blocks[i])
                emit_ctx_out(i - 3, blocks[i - 3])
            if n > 2:
                emit_ctx_out(n - 3, blocks[n - 3])
            # drain: interleave the last two blocks' ctx and out-projection
            # so each out-projection's ctx copy completes under the other
            # block's matmuls instead of stalling the PE
            if n > 1:
                emit_ctx(n - 2, blocks[-2])
                emit_ctx(n - 1, blocks[-1])
                emit_outproj_pair(n - 2, blocks[-2], n - 1, blocks[-1])
            else:
                emit_ctx_out(0, blocks[0])
            attp.release()
            xtp.release()

    nc.compile()
    return nc


def _prepare(X, masks, Wq, Wk, Wv, Wo):
    import ml_dtypes
    BF = ml_dtypes.bfloat16
    F8 = ml_dtypes.float8_e4m3

    X = np.asarray(X, dtype=np.float32)
    masks = np.asarray(masks)
    Wq = np.asarray(Wq, dtype=np.float32)
    Wk = np.asarray(Wk, dtype=np.float32)
    Wv = np.asarray(Wv, dtype=np.float32)
    Wo = np.asarray(Wo, dtype=np.float32)

    idxs = [np.where(masks[b] != 0)[0] for b in range(B)]
    # 256-multiples so fp8 DoubleRow k-tile pairs align for both batches
    LQs = [max(256, int(-(-len(ix) // 256) * 256)) for ix in idxs]
    LT = sum(LQs)
    offs = [0, LQs[0]]
    QBK = 512
    NBLK = -(-LT // QBK)
    LT_pad = NBLK * QBK
    n_dblk = D // 128

    # compacted, transposed X: columns = valid tokens (zero-padded)
    XTc = np.zeros((D, LT_pad), dtype=np.float32)
    for b in range(B):
        XTc[:, offs[b]:offs[b] + len(idxs[b])] = X[b].T[:, idxs[b]]
    # DMA-friendly: [NBLK, 128, n_dblk*QBK], per-partition contiguous
    XTa = np.ascontiguousarray(
        XTc.reshape(n_dblk, 128, NBLK, QBK).transpose(2, 1, 0, 3)
        .reshape(NBLK, 128, n_dblk * QBK)).astype(BF)

    caus = (np.arange(896)[None, :] - 384 >= np.arange(128)[:, None])

    nc = _build_program(LQs)

    def warr(wT):  # [D, JW] -> [128, n_dblk*JW] per-partition contiguous
        return np.ascontiguousarray(
            wT.reshape(n_dblk, 128, JW).transpose(1, 0, 2)
            .reshape(128, n_dblk * JW)).astype(BF)

    in_maps = []
    for c in range(NCORES):
        jsl = slice(c * JW, (c + 1) * JW)
        ind = np.zeros((JW, NH), dtype=np.float32)
        for h in range(NH):
            ind[h * DH:(h + 1) * DH, h] = 1.0
        in_maps.append({
            "XT": XTa,
            "WALL": np.ascontiguousarray(np.concatenate(
                [warr(Wq[jsl, :].T), warr(Wk[jsl, :].T),
                 warr(Wv[jsl, :].T)], axis=1)),
            "WOT": np.ascontiguousarray(Wo[:, jsl].T).astype(BF),
            "CAUS": caus.astype(BF),
            "IND": ind.astype(BF),
            "INDT": np.ascontiguousarray(ind.T).astype(BF),
        })

    return nc, in_maps, (idxs, LQs, LT, offs)


def _unshard(results, meta):
    idxs, LQs, LT, offs = meta
    n_dblk = D // 128
    blocks = _block_order(LQs)  # must match the device emission order

    partial = np.zeros((D, LT), dtype=np.float64)
    for c in range(NCORES):
        # OUTT[i, p, g*QB + w] = out[g*128 + p, ob + q0 + w] for block i
        ot = results[c]["OUTT"].astype(np.float64).reshape(
            len(blocks), 128, n_dblk, QB)
        for i, (b, q0, qw) in enumerate(blocks):
            cols = slice(offs[b] + q0, offs[b] + q0 + qw)
            partial[:, cols] += ot[i, :, :, :qw].transpose(1, 0, 2).reshape(
                D, qw)
    partial = partial.T  # [LT, D]

    out = np.zeros((B, S, D), dtype=np.float32)
    for b in range(B):
        out[b, idxs[b], :] = partial[offs[b]:offs[b] + len(idxs[b]), :].astype(
            np.float32)
    return out


def kernel(X, masks, Wq, Wk, Wv, Wo):
    from concourse.bass_utils import run_bass_kernel_spmd

    nc, in_maps, meta = _prepare(X, masks, Wq, Wk, Wv, Wo)
    res = run_bass_kernel_spmd(nc, in_maps, list(range(NCORES)))
    return _unshard(res.results, meta)


def profile_run(inputs, tmpdir=None):
    """Used by test.py: same program, run with NTFF tracing enabled."""
    from concourse.bass_utils import run_bass_kernel_spmd

    nc, in_maps, meta = _prepare(**inputs)
    res = run_bass_kernel_spmd(nc, in_maps, list(range(NCORES)), trace=True,
                               tmpdir=tmpdir)
    res.output = _unshard(res.results, meta)
    return res

